# revision 4
# baseline (speedup 1.0000x reference)
"""BiAffineParser span-classifier kernel for 8 Trainium2 NeuronCores.

Computes logits[b,i,j,n] = gelu(xs_proj[b,i] + xe_proj[b,j] + b1) @ W2 + b2
for the full L x L span grid without materializing the (B,L,L,H) tensor in
HBM.  Sharding: 8 cores = 4 batches x 2 halves of the i axis; each core
produces a (128, 256, 13) output shard.

Per-core dataflow (H=768 split into 6 chunks of 128 partitions):
  PE   : xsT/xeT projections (fp32), then W2 contraction with the gelu tile
         as the stationary operand (bf16) so output lands j-major in PSUM.
  DVE  : broadcast-add xeT[h,j] + (xsT+b1)[h,i] in bf16 (4x mode).
  ACT  : exact-erf Gelu on [128, 8192] tiles (the throughput bottleneck).
  DMA  : HWDGE loads; strided store of [j, (i,n)] tiles to the output.
"""

import os
import sys

if "/opt/trn_rl_repo" not in sys.path:
    sys.path.insert(0, "/opt/trn_rl_repo")

import numpy as np

B = 4
L = 256
H = 768
NH = 6           # 128-partition chunks of H
NL = 13          # num labels
IH = 128         # i rows per core
G = 32           # i-group size for the steady-state pipeline
NGRP = IH // G   # groups per core
GC = NL * G      # psum columns per j-tile per group

_CACHE = {}


def _build(repeat=1):
    import concourse.mybir as mybir
    from concourse import bacc
    from concourse.tile import TileContext

    f32 = mybir.dt.float32
    bf16 = mybir.dt.bfloat16
    GELU = mybir.ActivationFunctionType.Gelu

    nc = bacc.Bacc("TRN2", target_bir_lowering=False)

    xt_d = nc.dram_tensor("xt", [H, L], f32, kind="ExternalInput")
    xts_d = nc.dram_tensor("xts", [H, IH], f32, kind="ExternalInput")
    w1s_d = nc.dram_tensor("w1s", [H, H], f32, kind="ExternalInput")
    w1e_d = nc.dram_tensor("w1e", [H, H], f32, kind="ExternalInput")
    b1t_d = nc.dram_tensor("b1t", [128, NH], f32, kind="ExternalInput")
    w2_d = nc.dram_tensor("w2", [H, NL], f32, kind="ExternalInput")
    b2t_d = nc.dram_tensor("b2t", [128, GC], f32, kind="ExternalInput")
    out_d = nc.dram_tensor("out", [IH, L, NL], f32, kind="ExternalOutput")

    with TileContext(nc) as tc:
        def body():
            with (
                tc.tile_pool(name="consts", bufs=1) as cp,
                tc.tile_pool(name="pp", bufs=2, space="PSUM") as pp,
                tc.tile_pool(name="sump", bufs=2) as sp,
                tc.tile_pool(name="gelp", bufs=3) as gp,
                tc.tile_pool(name="outp", bufs=3) as op,
                tc.tile_pool(name="w1p", bufs=1) as wp,
            ):
                XT, XTS, W2B, XE, XSB = [], [], [], [], []
                for h in range(NH):
                    t = cp.tile([128, L], f32, tag=f"xt{h}", name=f"XT{h}")
                    nc.sync.dma_start(out=t, in_=xt_d[h * 128:(h + 1) * 128, :])
                    XT.append(t)
                    t = cp.tile([128, IH], f32, tag=f"xts{h}", name=f"XTS{h}")
                    nc.sync.dma_start(out=t, in_=xts_d[h * 128:(h + 1) * 128, :])
                    XTS.append(t)
                    t = cp.tile([128, NL], bf16, tag=f"w2b{h}", name=f"W2B{h}")
                    nc.gpsimd.dma_start(out=t, in_=w2_d[h * 128:(h + 1) * 128, :])
                    W2B.append(t)
                    XE.append(cp.tile([128, L], bf16, tag=f"xe{h}", name=f"XE{h}"))
                    XSB.append(cp.tile([128, IH], f32, tag=f"xsb{h}", name=f"XSB{h}"))
                B1T = cp.tile([128, NH], f32, tag="b1t", name="B1T")
                nc.sync.dma_start(out=B1T, in_=b1t_d[:, :])
                B2T = cp.tile([128, GC], f32, tag="b2t", name="B2T")
                nc.sync.dma_start(out=B2T, in_=b2t_d[:, :])

                W1S, W1E = [], []
                for h in range(NH):
                    t = wp.tile([128, H], f32, tag=f"w1s{h}", name=f"W1S{h}")
                    nc.sync.dma_start(out=t, in_=w1s_d[h * 128:(h + 1) * 128, :])
                    W1S.append(t)
                    t = wp.tile([128, H], f32, tag=f"w1e{h}", name=f"W1E{h}")
                    nc.sync.dma_start(out=t, in_=w1e_d[h * 128:(h + 1) * 128, :])
                    W1E.append(t)

                # Projections: xeT[k,:] over all L columns, xsT[k,:] over this
                # core's IH columns, b1 folded into xs.  fp32 matmuls.
                for k in range(NH):
                    pxe = pp.tile([128, L], f32, tag="pxe", name=f"pxe{k}")
                    for h in range(NH):
                        nc.tensor.matmul(
                            pxe,
                            lhsT=W1E[h][:, k * 128:(k + 1) * 128],
                            rhs=XT[h],
                            start=(h == 0),
                            stop=(h == NH - 1),
                        )
                    nc.vector.tensor_copy(out=XE[k], in_=pxe)
                    pxs = pp.tile([128, IH], f32, tag="pxs", name=f"pxs{k}")
                    for h in range(NH):
                        nc.tensor.matmul(
                            pxs,
                            lhsT=W1S[h][:, k * 128:(k + 1) * 128],
                            rhs=XTS[h],
                            start=(h == 0),
                            stop=(h == NH - 1),
                        )
                    nc.vector.tensor_scalar_add(
                        out=XSB[k], in0=pxs, scalar1=B1T[:, k:k + 1]
                    )

                # Steady state over i-groups.
                for g in range(NGRP):
                    gel = []
                    for c in range(NH):
                        st = sp.tile([128, G * L], bf16, tag="sum", name=f"sum{g}_{c}")
                        for il in range(G):
                            i = g * G + il
                            nc.vector.tensor_scalar_add(
                                out=st[:, il * L:(il + 1) * L],
                                in0=XE[c],
                                scalar1=XSB[c][:, i:i + 1],
                            )
                        gt = gp.tile([128, G * L], bf16, tag="gel", name=f"gel{g}_{c}")
                        nc.scalar.activation(out=gt, in_=st, func=GELU)
                        gel.append(gt)
                    ps = [
                        pp.tile([128, GC], f32, tag=f"ps{jt}", name=f"ps{g}_{jt}")
                        for jt in range(2)
                    ]
                    # PSUM has_written clears at BANK granularity on start=True,
                    # so exactly one start per psum tile: the very first MM.
                    # start=False into a cleared region overwrites-and-sets-bit.
                    for c in range(NH):
                        for il in range(G):
                            for jt in range(2):
                                nc.tensor.matmul(
                                    ps[jt][:, il * NL:(il + 1) * NL],
                                    lhsT=gel[c][:, il * L + jt * 128: il * L + jt * 128 + 128],
                                    rhs=W2B[c],
                                    start=(c == 0 and il == 0),
                                    stop=(c == NH - 1 and il == G - 1),
                                    skip_group_check=True,
                                )
                    for jt in range(2):
                        ob = op.tile([128, GC], f32, tag="ob", name=f"ob{g}_{jt}")
                        nc.vector.tensor_add(out=ob, in0=ps[jt], in1=B2T)
                        ov = out_d[
                            g * G:(g + 1) * G, jt * 128:(jt + 1) * 128, :
                        ].rearrange("i j n -> j i n")
                        nc.sync.dma_start(
                            out=ov, in_=ob.rearrange("p (i n) -> p i n", n=NL)
                        )

        if repeat == 1:
            body()
        else:
            with tc.For_i(0, repeat, 1):
                body()

    nc.compile()
    return nc


def _get_program(repeat=1):
    if repeat not in _CACHE:
        _CACHE[repeat] = _build(repeat)
    return _CACHE[repeat]


def make_in_maps(hidden_states, W1, b1, W2, b2):
    hidden_states = np.asarray(hidden_states, dtype=np.float32)
    W1 = np.asarray(W1, dtype=np.float32)
    b1 = np.asarray(b1, dtype=np.float32)
    W2 = np.asarray(W2, dtype=np.float32)
    b2 = np.asarray(b2, dtype=np.float32)

    w1s = np.ascontiguousarray(W1[:H])
    w1e = np.ascontiguousarray(W1[H:])
    b1t = np.ascontiguousarray(b1.reshape(NH, 128).T)
    b2t = np.ascontiguousarray(np.tile(b2, (128, G)))

    in_maps = []
    for core in range(8):
        b, ih = core // 2, core % 2
        xt = np.ascontiguousarray(hidden_states[b].T)
        xts = np.ascontiguousarray(xt[:, ih * IH:(ih + 1) * IH])
        in_maps.append(
            {
                "xt": xt,
                "xts": xts,
                "w1s": w1s,
                "w1e": w1e,
                "b1t": b1t,
                "w2": W2,
                "b2t": b2t,
            }
        )
    return in_maps


def kernel(hidden_states, W1, b1, W2, b2):
    from concourse.bass_utils import run_bass_kernel_spmd

    nc = _get_program()
    in_maps = make_in_maps(hidden_states, W1, b1, W2, b2)
    res = run_bass_kernel_spmd(nc, in_maps, core_ids=list(range(8)))

    out = np.empty((B, L, L, NL), dtype=np.float32)
    for core in range(8):
        b, ih = core // 2, core % 2
        out[b, ih * IH:(ih + 1) * IH] = res.results[core]["out"]
    return out
